# revision 16
# baseline (speedup 1.0000x reference)
"""Trainium2 Bass kernel for the BiLSTM pair-scoring model.

Data-parallel over 8 NeuronCores: each core runs 64 of the 512 sequences
(both LSTM directions) fully on-device: embedding gather (indirect DMA with
f32->bf16 cast, GK windows per instr), PE transpose to hidden-major,
bidirectional LSTM scan, masked mean, MLP head, sigmoid.

v2 recurrence: all four gate nonlinearities are computed by a SINGLE Tanh
activation per step per direction using sigmoid(x) = (tanh(x/2)+1)/2:
  - ACT applies tanh(0.5*z) to the whole 4-gate PSUM slab (scale=0.5).
  - j columns of Wx/Wh are pre-doubled so slot j yields tanh(z_j) exactly.
  - cell state is tracked doubled (D = 2c) and hidden doubled (H = 2h):
      u1 = (tau_f + 1) * D          [DVE STT]
      u2 = (tau_i + 1) * tau_j      [GpSimd STT]
      D' = 0.5*u1 + u2              [DVE STT]   (= 2c')
      tau_c = tanh(0.5*D')          [ACT]       (= tanh(c'))
      H  = (tau_o + 1) * tau_c      [DVE STT]   (= 2h)
    Wh is pre-halved (H is doubled) and W_mid absorbs the extra 1/2.
  - forget bias +1.0 and the o-gate -1e9 mask are rank-1 matmuls sharing
    one all-ones lhsT column; the mask row is preloaded in full.
  - mean over t: identity-matmul PSUM accumulation batched 8 steps at a
    time from an 8-slot H history ring (one LDWEIGHTS per 8 steps).
"""

import sys

for p in ("/opt/trn_rl_repo", "/root/.axon_site/_ro/trn_rl_repo"):
    if p not in sys.path:
        sys.path.insert(0, p)

import numpy as np

VOCAB = 200000
E = 128
H = 128
OH = 1024
B = 256
L = 256
NCORES = 8
G = 64          # sequences per core
W = 2           # recurrence steps per PSUM window
NW = L // W     # 64 windows
P = 128
GK = 1          # windows gathered per indirect-DMA instruction
AB = 8          # steps per mean-accumulation batch

# psum slot order: slot0=i(ref0), slot1=o(ref3), slot2=f(ref2), slot3=j(ref1)
_SLOT_TO_REF = {0: 0, 1: 3, 2: 2, 3: 1}


def _build_graph(any_mask: bool, b_out_val: float):
    import concourse.bass as bass
    import concourse.mybir as mybir
    from concourse import bacc
    from concourse.masks import make_identity
    from concourse.tile import TileContext

    f32 = mybir.dt.float32
    bf16 = mybir.dt.bfloat16
    i32 = mybir.dt.int32
    AF = mybir.ActivationFunctionType
    OP = mybir.AluOpType

    nc = bacc.Bacc("TRN2", target_bir_lowering=False)

    # ---- DRAM IO ----
    emb_d = nc.dram_tensor("emb", [VOCAB, E], f32, kind="ExternalInput")
    ids_d = nc.dram_tensor("ids", [2 * (NW // GK) * P, GK], i32, kind="ExternalInput")
    wx_d = nc.dram_tensor("wx", [P, 2 * 4 * H], bf16, kind="ExternalInput")
    wh_d = nc.dram_tensor("wh", [P, 2 * 4 * H], bf16, kind="ExternalInput")
    # per (dir, window) rank-1 row over the full 4-slot z tile:
    # f-slot = +1.0 (forget bias), o-slot = -1e9*mask, i/j = 0
    om_d = nc.dram_tensor("omask", [1, NW * 4 * W * G], bf16, kind="ExternalInput")
    wmid_d = nc.dram_tensor("wmid", [P, 4 * OH], f32, kind="ExternalInput")
    bmid_d = nc.dram_tensor("bmid", [P, 8], f32, kind="ExternalInput")
    wout_d = nc.dram_tensor("wout", [P, 8], f32, kind="ExternalInput")
    out_d = nc.dram_tensor("out", [1, G // 2], f32, kind="ExternalOutput")

    with TileContext(nc) as tc:
        with (
            tc.tile_pool(name="const", bufs=1) as cpool,
            tc.tile_pool(name="state", bufs=1) as spool,
            tc.tile_pool(name="gath", bufs=8) as gpool,
            tc.tile_pool(name="act", bufs=3) as apool,
        ):
            # ---- constants / weights to SBUF ----
            NGg = NW // GK
            ids_tiles = {}
            for dd in range(2):
                for gg in range(NGg):
                    it = cpool.tile([P, GK], i32, name=f"idsg{dd}_{gg}")
                    r0 = (dd * NGg + gg) * P
                    nc.sync.dma_start(out=it[:], in_=ids_d[r0 : r0 + P, :])
                    ids_tiles[(dd, gg)] = it
            wx_sb = cpool.tile([P, 2 * 4 * H], bf16)
            nc.sync.dma_start(out=wx_sb[:], in_=wx_d[:])
            wh_sb = cpool.tile([P, 2 * 4 * H], bf16)
            nc.sync.dma_start(out=wh_sb[:], in_=wh_d[:])
            wmid_sb = cpool.tile([P, 4 * OH], f32)
            nc.sync.dma_start(out=wmid_sb[:], in_=wmid_d[:])
            bmid_sb = cpool.tile([P, 8], f32)
            nc.sync.dma_start(out=bmid_sb[:], in_=bmid_d[:])
            wout_sb = cpool.tile([P, 8], f32)
            nc.sync.dma_start(out=wout_sb[:], in_=wout_d[:])
            if any_mask:
                om_sb = cpool.tile([1, NW * 4 * W * G], bf16)
                nc.sync.dma_start(out=om_sb[:], in_=om_d[:])
            else:
                # constant rank-1 row: +1.0 on the f slot only
                om_sb = cpool.tile([1, 4 * W * G], bf16)
                nc.vector.memset(om_sb[:], 0.0)
                nc.vector.memset(om_sb[:, 2 * W * G : 3 * W * G], 1.0)
            ident = cpool.tile([P, P], bf16)
            make_identity(nc, ident[:])
            ident32 = cpool.tile([P, P], f32)
            make_identity(nc, ident32[:])
            ones_col = cpool.tile([1, P], bf16)
            nc.vector.memset(ones_col[:], 1.0)

            # ---- LSTM state: D = 2c (f32); H history ring (bf16, = 2h) ----
            d0 = spool.tile([P, G], f32)
            d1 = spool.tile([P, G], f32)
            dts = [d0, d1]
            hh0 = spool.tile([P, AB * G], bf16)
            hh1 = spool.tile([P, AB * G], bf16)
            hhs = [hh0, hh1]
            nc.vector.memset(d0[:], 0.0)
            nc.vector.memset(d1[:], 0.0)
            nc.vector.memset(hh0[:], 0.0)
            nc.vector.memset(hh1[:], 0.0)

            # Full-resident xT buffer (transposed embeddings)
            xc_all = spool.tile([P, 2 * NW * W * G], bf16)   # 32 KiB/part
            LOOKG = 6  # gather lookahead in gather-groups (GK windows each)

            with (
                tc.tile_pool(name="psz0", bufs=2, space="PSUM") as zpool0,
                tc.tile_pool(name="psz1", bufs=2, space="PSUM") as zpool1,
                tc.tile_pool(name="pst0", bufs=1, space="PSUM") as tpool0,
                tc.tile_pool(name="pst1", bufs=1, space="PSUM") as tpool1,
                tc.tile_pool(name="psacc", bufs=1, space="PSUM") as accpool,
            ):
                acc_ps = accpool.tile([P, 2 * G], f32)
                gtiles = {}

                NG = NW // GK  # gather groups per direction

                def issue_gather(g_):
                    for d_ in range(2):
                        gt = gpool.tile([P, GK * P], bf16, tag=f"gt{d_}",
                                        name=f"gt{d_}_{g_}")
                        nc.gpsimd.indirect_dma_start(
                            out=gt[:],
                            out_offset=None,
                            in_=emb_d[:],
                            in_offset=bass.IndirectOffsetOnAxis(
                                ap=ids_tiles[(d_, g_)][:], axis=0
                            ),
                        )
                        gtiles[(d_, g_)] = gt

                for g_ in range(min(LOOKG, NG)):
                    issue_gather(g_)

                for w in range(NW):
                    gi, gk = divmod(w, GK)
                    if gk == 0 and gi + LOOKG < NG:
                        issue_gather(gi + LOOKG)
                    # -- PE transpose of gathered window tiles into xT --
                    xts = []
                    for d in range(2):
                        xc = xc_all[:, (d * NW + w) * W * G : (d * NW + w + 1) * W * G]
                        pt = (tpool0 if d == 0 else tpool1).tile(
                            [P, P], bf16, tag="pt"
                        )
                        gt = gtiles[(d, gi)]
                        nc.tensor.transpose(
                            out=pt[:], in_=gt[:, gk * P : (gk + 1) * P],
                            identity=ident[:],
                        )
                        nc.vector.tensor_copy(xc[:], pt[:])
                        if gk == GK - 1:
                            gtiles.pop((d, gi))
                        xts.append(xc)

                    # -- x-part matmuls into PSUM (weight-stationary) --
                    zt0 = zpool0.tile([P, 4 * W * G], f32, tag="zt0", name=f"zt0_{w}")
                    zt1 = zpool1.tile([P, 4 * W * G], f32, tag="zt1", name=f"zt1_{w}")
                    zts = [zt0, zt1]
                    for d in range(2):
                        zt = zts[d]
                        for slot in range(4):
                            lhsT = wx_sb[:, d * 512 + slot * H : d * 512 + (slot + 1) * H]
                            outap = zt[:, slot * W * G : (slot + 1) * W * G]
                            nc.tensor.matmul(
                                out=outap, lhsT=lhsT, rhs=xts[d],
                                start=True, stop=False,
                            )
                        # rank-1: +1.0 into the f-gate slot (forget bias)
                        base = w * 4 * W * G if any_mask else 0
                        nc.tensor.matmul(
                            out=zt[:, 2 * W * G : 3 * W * G],
                            lhsT=ones_col[:1, :],
                            rhs=om_sb[:1, base + 2 * W * G : base + 3 * W * G],
                            start=False, stop=False,
                            skip_group_check=True,
                        )
                        if any_mask:
                            # rank-1: (-1e9*mask01) into the o-gate slot
                            nc.tensor.matmul(
                                out=zt[:, 1 * W * G : 2 * W * G],
                                lhsT=ones_col[:1, :],
                                rhs=om_sb[:1, base + 1 * W * G : base + 2 * W * G],
                                start=False, stop=False,
                                skip_group_check=True,
                            )

                    # -- W recurrence steps, two per-dir chains --
                    for tt in range(W):
                        t_glob = w * W + tt
                        cur = t_glob % AB
                        prv = (t_glob + AB - 1) % AB
                        for d in range(2):
                            zt = zts[d]
                            hh = hhs[d]
                            for slot in range(4):
                                lhsT = wh_sb[:, d * 512 + slot * H
                                             : d * 512 + (slot + 1) * H]
                                outap = zt[:, slot * W * G + tt * G
                                           : slot * W * G + (tt + 1) * G]
                                nc.tensor.matmul(
                                    out=outap, lhsT=lhsT,
                                    rhs=hh[:, prv * G : (prv + 1) * G],
                                    start=False, stop=(tt == W - 1),
                                    skip_group_check=True,
                                )

                            z_v = zt[:].rearrange(
                                "p (g t s) -> p g t s", g=4, t=W, s=G
                            )
                            # one Tanh over all four gate slots of step tt
                            tau = apool.tile([P, 4 * G], f32, tag=f"tau{d}")
                            tau_v = tau[:].rearrange("p (g s) -> p g s", g=4)
                            nc.scalar.activation(
                                tau_v, z_v[:, 0:4, tt, :], AF.Tanh, scale=0.5
                            )
                            u1 = apool.tile([P, G], f32, tag=f"u1{d}")
                            u2 = apool.tile([P, G], f32, tag=f"u2{d}")
                            tc_t = apool.tile([P, G], f32, tag=f"tc{d}")
                            ds = dts[d][:]

                            # u2 = (tau_i + 1) * tau_j
                            nc.vector.scalar_tensor_tensor(
                                out=u2[:], in0=tau_v[:, 0, :], scalar=1.0,
                                in1=tau_v[:, 3, :], op0=OP.add, op1=OP.mult,
                            )
                            # u1 = (tau_f + 1) * D
                            nc.vector.scalar_tensor_tensor(
                                out=u1[:], in0=tau_v[:, 2, :], scalar=1.0,
                                in1=ds, op0=OP.add, op1=OP.mult,
                            )
                            # D' = 0.5*u1 + u2
                            nc.vector.scalar_tensor_tensor(
                                out=ds, in0=u1[:], scalar=0.5,
                                in1=u2[:], op0=OP.mult, op1=OP.add,
                            )
                            # tau_c = tanh(0.5*D')
                            nc.scalar.activation(tc_t[:], ds, AF.Tanh, scale=0.5)
                            # H = (tau_o + 1) * tau_c  -> bf16 ring slot
                            nc.vector.scalar_tensor_tensor(
                                out=hh[:, cur * G : (cur + 1) * G],
                                in0=tau_v[:, 1, :], scalar=1.0,
                                in1=tc_t[:], op0=OP.add, op1=OP.mult,
                            )
                        if cur == AB - 1:
                            # acc += sum of the 8 H slots (identity matmuls,
                            # one LDWEIGHTS per batch)
                            first = t_glob == AB - 1
                            last = t_glob == L - 1
                            for d in range(2):
                                hh = hhs[d]
                                for k in range(AB):
                                    nc.tensor.matmul(
                                        out=acc_ps[:, d * G : (d + 1) * G],
                                        lhsT=ident[:],
                                        rhs=hh[:, k * G : (k + 1) * G],
                                        start=(first and k == 0),
                                        stop=(last and k == AB - 1),
                                        skip_group_check=True,
                                    )

            # ---- MLP head (recurrence PSUM pools closed; banks free) ----
            with (
                tc.tile_pool(name="psm", bufs=2, space="PSUM") as mpool,
                tc.tile_pool(name="psl", bufs=1, space="PSUM") as lpool,
            ):
                npair = G // 2  # 32
                feats = cpool.tile([P, 4 * npair], f32)
                zeros32 = cpool.tile([P, npair], f32)
                nc.vector.memset(zeros32[:], 0.0)
                for k, (didx, par) in enumerate([(0, 0), (1, 0), (0, 1), (1, 1)]):
                    asrc = acc_ps[:].rearrange("p (d s2 two) -> p d s2 two", d=2, two=2)
                    nc.vector.tensor_copy(
                        feats[:, k * npair : (k + 1) * npair],
                        asrc[:, didx, :, par],
                    )
                logit_ps = lpool.tile([1, npair], f32)
                for j in range(8):
                    hps = mpool.tile([P, npair], f32, tag="hps")
                    for k in range(4):
                        nc.tensor.matmul(
                            out=hps[:],
                            lhsT=wmid_sb[:, k * OH + j * P : k * OH + (j + 1) * P],
                            rhs=feats[:, k * npair : (k + 1) * npair],
                            start=(k == 0), stop=(k == 3),
                        )
                    # relu(x + b) on DVE: (hps + bmid_j) max 0
                    hid = apool.tile([P, npair], f32, tag="hid")
                    nc.vector.scalar_tensor_tensor(
                        out=hid[:], in0=hps[:], scalar=bmid_sb[:, j : j + 1],
                        in1=zeros32[:], op0=OP.add, op1=OP.max,
                    )
                    nc.tensor.matmul(
                        out=logit_ps[:],
                        lhsT=wout_sb[:, j : j + 1],
                        rhs=hid[:],
                        start=(j == 0), stop=(j == 7),
                        skip_group_check=True,
                    )
                out_sb = cpool.tile([1, npair], f32)
                nc.scalar.activation(
                    out_sb[:], logit_ps[:], AF.Sigmoid, bias=float(b_out_val)
                )
                nc.sync.dma_start(out=out_d[:], in_=out_sb[:])

    if not nc.is_finalized():
        nc.finalize()
    return nc


def _host_prep(s1, s2, emb_W, W_fwd, b_fwd, W_bwd, b_bwd, W_mid, b_mid, W_out, b_out):
    import ml_dtypes

    bf = ml_dtypes.bfloat16
    s1 = np.asarray(s1); s2 = np.asarray(s2)
    inp = np.concatenate([s1, s2], axis=1).reshape(-1, L).astype(np.int32)  # [512, L]
    lens = (inp != 0).sum(axis=1).astype(np.int32)                          # [512]
    t = np.arange(L)[None, :]
    ridx = np.where(t < lens[:, None], lens[:, None] - 1 - t, t)
    rev = np.take_along_axis(inp, ridx, axis=1)                             # [512, L]

    any_mask = bool((lens < L).any())
    emb = np.ascontiguousarray(np.asarray(emb_W, dtype=np.float32))

    # weights shared by all cores; tanh-form scaling:
    #   sigma slots (i,o,f): wx *1, wh *0.5 (H doubled)
    #   j slot: pre-doubled -> wx *2, wh *1.0
    sx = {0: 1.0, 1: 1.0, 2: 1.0, 3: 2.0}
    sh = {0: 0.5, 1: 0.5, 2: 0.5, 3: 1.0}
    wx = np.empty((P, 2 * 4 * H), dtype=np.float32)
    wh = np.empty((P, 2 * 4 * H), dtype=np.float32)
    for d, Wd in enumerate((W_fwd, W_bwd)):
        Wd = np.asarray(Wd, dtype=np.float32)
        for slot in range(4):
            ref = _SLOT_TO_REF[slot]
            cols = slice(ref * H, (ref + 1) * H)
            wx[:, d * 512 + slot * H : d * 512 + (slot + 1) * H] = Wd[:E, cols] * sx[slot]
            wh[:, d * 512 + slot * H : d * 512 + (slot + 1) * H] = Wd[E:, cols] * sh[slot]
    wx = wx.astype(bf)
    wh = wh.astype(bf)

    # fold mean /L and the H-doubling /2 into W_mid
    Wm = np.asarray(W_mid, dtype=np.float32) / float(2 * L)
    wmid = np.empty((P, 4 * OH), dtype=np.float32)
    for k in range(4):
        wmid[:, k * OH : (k + 1) * OH] = Wm[k * P : (k + 1) * P, :]
    bmid = np.asarray(b_mid, dtype=np.float32).reshape(8, P).T.copy()
    wout = np.asarray(W_out, dtype=np.float32).reshape(8, P).T.copy()

    in_maps = []
    for c in range(NCORES):
        rows = slice(c * G, (c + 1) * G)
        ids = np.empty((P, 2 * NW), dtype=np.int32)
        for d, arr in enumerate((inp[rows], rev[rows])):
            tiles = arr.T.reshape(NW, W * G)  # [tile, 128]
            ids[:, d * NW : (d + 1) * NW] = tiles.T
        NGg = NW // GK
        idsg = np.empty((2 * NGg * P, GK), dtype=np.int32)
        for d in range(2):
            for g in range(NGg):
                blk = ids[:, d * NW + g * GK : d * NW + (g + 1) * GK]
                idsg[(d * NGg + g) * P : (d * NGg + g + 1) * P, :] = blk
        lcore = lens[rows]
        om = (np.arange(L)[:, None] >= lcore[None, :]).astype(np.float32)  # [L, G]
        om4 = om.reshape(NW, W * G) * np.float32(-1e9)   # o-slot rows per window
        # full rank-1 row per (dir, window): [i=0 | o=-1e9*mask | f=1 | j=0]
        rowz = np.zeros((NW, 4, W * G), dtype=np.float32)
        rowz[:, 1, :] = om4
        rowz[:, 2, :] = 1.0
        omask = rowz.reshape(1, NW * 4 * W * G).astype(bf)  # shared by both dirs
        in_maps.append({
            "emb": emb, "ids": idsg, "wx": wx, "wh": wh, "omask": omask,
            "wmid": wmid, "bmid": bmid, "wout": wout,
        })
    assert not np.any(np.asarray(b_fwd)) and not np.any(np.asarray(b_bwd)), \
        "nonzero LSTM biases not supported by this kernel build"
    return in_maps, any_mask, float(np.asarray(b_out).reshape(-1)[0])


_CACHE = {}


def kernel(**inputs):
    from concourse import bass_utils

    in_maps, any_mask, b_out_val = _host_prep(**inputs)
    key = ("g", any_mask, b_out_val)
    if key not in _CACHE:
        _CACHE[key] = _build_graph(any_mask, b_out_val)
    nc = _CACHE[key]
    res = bass_utils.run_bass_kernel_spmd(
        nc, in_maps, core_ids=list(range(NCORES))
    )
    outs = [np.asarray(res.results[c]["out"]).reshape(-1) for c in range(NCORES)]
    return np.concatenate(outs).astype(np.float32)


# revision 17
# speedup vs baseline: 1.1602x; 1.1602x over previous
"""Trainium2 Bass kernel for the BiLSTM pair-scoring model.

Data-parallel over 8 NeuronCores: each core runs 64 of the 512 sequences
(both LSTM directions) fully on-device: embedding gather (indirect DMA with
f32->bf16 cast, GK windows per instr), PE transpose to hidden-major,
bidirectional LSTM scan, masked mean, MLP head, sigmoid.

v2 recurrence: all four gate nonlinearities are computed by a SINGLE Tanh
activation per step per direction using sigmoid(x) = (tanh(x/2)+1)/2:
  - ACT applies tanh(0.5*z) to the whole 4-gate PSUM slab (scale=0.5).
  - j columns of Wx/Wh are pre-doubled so slot j yields tanh(z_j) exactly.
  - cell state is tracked doubled (D = 2c) and hidden doubled (H = 2h):
      u1 = (tau_f + 1) * D          [DVE STT]
      u2 = (tau_i + 1) * tau_j      [GpSimd STT]
      D' = 0.5*u1 + u2              [DVE STT]   (= 2c')
      tau_c = tanh(0.5*D')          [ACT]       (= tanh(c'))
      H  = (tau_o + 1) * tau_c      [DVE STT]   (= 2h)
    Wh is pre-halved (H is doubled) and W_mid absorbs the extra 1/2.
  - forget bias +1.0 and the o-gate -1e9 mask are rank-1 matmuls sharing
    one all-ones lhsT column; the mask row is preloaded in full.
  - mean over t: identity-matmul PSUM accumulation batched 8 steps at a
    time from an 8-slot H history ring (one LDWEIGHTS per 8 steps).
"""

import sys

for p in ("/opt/trn_rl_repo", "/root/.axon_site/_ro/trn_rl_repo"):
    if p not in sys.path:
        sys.path.insert(0, p)

import numpy as np

VOCAB = 200000
E = 128
H = 128
OH = 1024
B = 256
L = 256
NCORES = 8
G = 64          # sequences per core
W = 2           # recurrence steps per PSUM window
NW = L // W     # 64 windows
P = 128
GK = 1          # windows gathered per indirect-DMA instruction
AB = 8          # steps per mean-accumulation batch

# psum slot order: slot0=i(ref0), slot1=o(ref3), slot2=f(ref2), slot3=j(ref1)
_SLOT_TO_REF = {0: 0, 1: 3, 2: 2, 3: 1}


def _build_graph(any_mask: bool, b_out_val: float):
    import concourse.bass as bass
    import concourse.mybir as mybir
    from concourse import bacc
    from concourse.masks import make_identity
    from concourse.tile import TileContext

    f32 = mybir.dt.float32
    bf16 = mybir.dt.bfloat16
    i32 = mybir.dt.int32
    AF = mybir.ActivationFunctionType
    OP = mybir.AluOpType

    nc = bacc.Bacc("TRN2", target_bir_lowering=False)

    # ---- DRAM IO ----
    emb_d = nc.dram_tensor("emb", [VOCAB, E], f32, kind="ExternalInput")
    ids_d = nc.dram_tensor("ids", [P, 2 * NW], i32, kind="ExternalInput")
    wx_d = nc.dram_tensor("wx", [P, 2 * 4 * H], bf16, kind="ExternalInput")
    wh_d = nc.dram_tensor("wh", [P, 2 * 4 * H], bf16, kind="ExternalInput")
    # per (dir, window) rank-1 row over the full 4-slot z tile:
    # f-slot = +1.0 (forget bias), o-slot = -1e9*mask, i/j = 0
    om_d = nc.dram_tensor("omask", [1, NW * 4 * W * G], bf16, kind="ExternalInput")
    wmid_d = nc.dram_tensor("wmid", [P, 4 * OH], f32, kind="ExternalInput")
    bmid_d = nc.dram_tensor("bmid", [P, 8], f32, kind="ExternalInput")
    wout_d = nc.dram_tensor("wout", [P, 8], f32, kind="ExternalInput")
    out_d = nc.dram_tensor("out", [1, G // 2], f32, kind="ExternalOutput")

    with TileContext(nc) as tc:
        with (
            tc.tile_pool(name="const", bufs=1) as cpool,
            tc.tile_pool(name="state", bufs=1) as spool,
            tc.tile_pool(name="gath", bufs=8) as gpool,
            tc.tile_pool(name="act", bufs=3) as apool,
        ):
            # ---- constants / weights to SBUF ----
            ids_sb = cpool.tile([P, 2 * NW], i32)
            nc.sync.dma_start(out=ids_sb[:], in_=ids_d[:])
            wx_sb = cpool.tile([P, 2 * 4 * H], bf16)
            nc.sync.dma_start(out=wx_sb[:], in_=wx_d[:])
            wh_sb = cpool.tile([P, 2 * 4 * H], bf16)
            nc.sync.dma_start(out=wh_sb[:], in_=wh_d[:])
            wmid_sb = cpool.tile([P, 4 * OH], f32)
            nc.sync.dma_start(out=wmid_sb[:], in_=wmid_d[:])
            bmid_sb = cpool.tile([P, 8], f32)
            nc.sync.dma_start(out=bmid_sb[:], in_=bmid_d[:])
            wout_sb = cpool.tile([P, 8], f32)
            nc.sync.dma_start(out=wout_sb[:], in_=wout_d[:])
            if any_mask:
                om_sb = cpool.tile([1, NW * 4 * W * G], bf16)
                nc.sync.dma_start(out=om_sb[:], in_=om_d[:])
            else:
                # constant rank-1 row: +1.0 on the f slot only
                om_sb = cpool.tile([1, 4 * W * G], bf16)
                nc.vector.memset(om_sb[:], 0.0)
                nc.vector.memset(om_sb[:, 2 * W * G : 3 * W * G], 1.0)
            ident = cpool.tile([P, P], bf16)
            make_identity(nc, ident[:])
            ident32 = cpool.tile([P, P], f32)
            make_identity(nc, ident32[:])
            ones_col = cpool.tile([1, P], bf16)
            nc.vector.memset(ones_col[:], 1.0)

            # ---- LSTM state: D = 2c (f32); H history ring (bf16, = 2h) ----
            d0 = spool.tile([P, G], f32)
            d1 = spool.tile([P, G], f32)
            dts = [d0, d1]
            hh0 = spool.tile([P, AB * G], bf16)
            hh1 = spool.tile([P, AB * G], bf16)
            hhs = [hh0, hh1]
            nc.vector.memset(d0[:], 0.0)
            nc.vector.memset(d1[:], 0.0)
            nc.vector.memset(hh0[:], 0.0)
            nc.vector.memset(hh1[:], 0.0)

            # Full-resident xT buffer (transposed embeddings)
            xc_all = spool.tile([P, 2 * NW * W * G], bf16)   # 32 KiB/part
            LOOKG = 6  # gather lookahead in gather-groups (GK windows each)

            with (
                tc.tile_pool(name="psz0", bufs=2, space="PSUM") as zpool0,
                tc.tile_pool(name="psz1", bufs=2, space="PSUM") as zpool1,
                tc.tile_pool(name="pst0", bufs=1, space="PSUM") as tpool0,
                tc.tile_pool(name="pst1", bufs=1, space="PSUM") as tpool1,
                tc.tile_pool(name="psacc", bufs=1, space="PSUM") as accpool,
            ):
                acc_ps = accpool.tile([P, 2 * G], f32)
                gtiles = {}

                NG = NW // GK  # gather groups per direction

                def issue_gather(g_):
                    for d_ in range(2):
                        gt = gpool.tile([P, GK * P], bf16, tag=f"gt{d_}",
                                        name=f"gt{d_}_{g_}")
                        nc.gpsimd.indirect_dma_start(
                            out=gt[:],
                            out_offset=None,
                            in_=emb_d[:],
                            in_offset=bass.IndirectOffsetOnAxis(
                                ap=ids_sb[:, d_ * NW + g_ * GK
                                          : d_ * NW + g_ * GK + GK], axis=0
                            ),
                        )
                        gtiles[(d_, g_)] = gt

                for g_ in range(min(LOOKG, NG)):
                    issue_gather(g_)

                for w in range(NW):
                    gi, gk = divmod(w, GK)
                    if gk == 0 and gi + LOOKG < NG:
                        issue_gather(gi + LOOKG)
                    # -- PE transpose of gathered window tiles into xT --
                    xts = []
                    for d in range(2):
                        xc = xc_all[:, (d * NW + w) * W * G : (d * NW + w + 1) * W * G]
                        pt = (tpool0 if d == 0 else tpool1).tile(
                            [P, P], bf16, tag="pt"
                        )
                        gt = gtiles[(d, gi)]
                        nc.tensor.transpose(
                            out=pt[:], in_=gt[:, gk * P : (gk + 1) * P],
                            identity=ident[:],
                        )
                        nc.vector.tensor_copy(xc[:], pt[:])
                        if gk == GK - 1:
                            gtiles.pop((d, gi))
                        xts.append(xc)

                    # -- x-part matmuls into PSUM (weight-stationary) --
                    zt0 = zpool0.tile([P, 4 * W * G], f32, tag="zt0", name=f"zt0_{w}")
                    zt1 = zpool1.tile([P, 4 * W * G], f32, tag="zt1", name=f"zt1_{w}")
                    zts = [zt0, zt1]
                    for d in range(2):
                        zt = zts[d]
                        for slot in range(4):
                            lhsT = wx_sb[:, d * 512 + slot * H : d * 512 + (slot + 1) * H]
                            outap = zt[:, slot * W * G : (slot + 1) * W * G]
                            nc.tensor.matmul(
                                out=outap, lhsT=lhsT, rhs=xts[d],
                                start=True, stop=False,
                            )
                        # rank-1: +1.0 into the f-gate slot (forget bias)
                        base = w * 4 * W * G if any_mask else 0
                        nc.tensor.matmul(
                            out=zt[:, 2 * W * G : 3 * W * G],
                            lhsT=ones_col[:1, :],
                            rhs=om_sb[:1, base + 2 * W * G : base + 3 * W * G],
                            start=False, stop=False,
                            skip_group_check=True,
                        )
                        if any_mask:
                            # rank-1: (-1e9*mask01) into the o-gate slot
                            nc.tensor.matmul(
                                out=zt[:, 1 * W * G : 2 * W * G],
                                lhsT=ones_col[:1, :],
                                rhs=om_sb[:1, base + 1 * W * G : base + 2 * W * G],
                                start=False, stop=False,
                                skip_group_check=True,
                            )

                    # -- W recurrence steps, two per-dir chains --
                    for tt in range(W):
                        t_glob = w * W + tt
                        cur = t_glob % AB
                        prv = (t_glob + AB - 1) % AB
                        for d in range(2):
                            zt = zts[d]
                            hh = hhs[d]
                            for slot in range(4):
                                lhsT = wh_sb[:, d * 512 + slot * H
                                             : d * 512 + (slot + 1) * H]
                                outap = zt[:, slot * W * G + tt * G
                                           : slot * W * G + (tt + 1) * G]
                                nc.tensor.matmul(
                                    out=outap, lhsT=lhsT,
                                    rhs=hh[:, prv * G : (prv + 1) * G],
                                    start=False, stop=(tt == W - 1),
                                    skip_group_check=True,
                                )

                            z_v = zt[:].rearrange(
                                "p (g t s) -> p g t s", g=4, t=W, s=G
                            )
                            # one Tanh over all four gate slots of step tt
                            tau = apool.tile([P, 4 * G], f32, tag=f"tau{d}")
                            tau_v = tau[:].rearrange("p (g s) -> p g s", g=4)
                            nc.scalar.activation(
                                tau_v, z_v[:, 0:4, tt, :], AF.Tanh, scale=0.5
                            )
                            u1 = apool.tile([P, G], f32, tag=f"u1{d}")
                            u2 = apool.tile([P, G], f32, tag=f"u2{d}")
                            tc_t = apool.tile([P, G], f32, tag=f"tc{d}")
                            ds = dts[d][:]

                            # u2 = (tau_i + 1) * tau_j
                            nc.vector.scalar_tensor_tensor(
                                out=u2[:], in0=tau_v[:, 0, :], scalar=1.0,
                                in1=tau_v[:, 3, :], op0=OP.add, op1=OP.mult,
                            )
                            # u1 = (tau_f + 1) * D
                            nc.vector.scalar_tensor_tensor(
                                out=u1[:], in0=tau_v[:, 2, :], scalar=1.0,
                                in1=ds, op0=OP.add, op1=OP.mult,
                            )
                            # D' = 0.5*u1 + u2
                            nc.vector.scalar_tensor_tensor(
                                out=ds, in0=u1[:], scalar=0.5,
                                in1=u2[:], op0=OP.mult, op1=OP.add,
                            )
                            # tau_c = tanh(0.5*D')
                            nc.scalar.activation(tc_t[:], ds, AF.Tanh, scale=0.5)
                            # H = (tau_o + 1) * tau_c  -> bf16 ring slot
                            nc.vector.scalar_tensor_tensor(
                                out=hh[:, cur * G : (cur + 1) * G],
                                in0=tau_v[:, 1, :], scalar=1.0,
                                in1=tc_t[:], op0=OP.add, op1=OP.mult,
                            )
                        if cur == AB - 1:
                            # acc += sum of the 8 H slots (identity matmuls,
                            # one LDWEIGHTS per batch)
                            first = t_glob == AB - 1
                            last = t_glob == L - 1
                            for d in range(2):
                                hh = hhs[d]
                                for k in range(AB):
                                    nc.tensor.matmul(
                                        out=acc_ps[:, d * G : (d + 1) * G],
                                        lhsT=ident[:],
                                        rhs=hh[:, k * G : (k + 1) * G],
                                        start=(first and k == 0),
                                        stop=(last and k == AB - 1),
                                        skip_group_check=True,
                                    )

            # ---- MLP head (recurrence PSUM pools closed; banks free) ----
            with (
                tc.tile_pool(name="psm", bufs=2, space="PSUM") as mpool,
                tc.tile_pool(name="psl", bufs=1, space="PSUM") as lpool,
            ):
                npair = G // 2  # 32
                feats = cpool.tile([P, 4 * npair], f32)
                zeros32 = cpool.tile([P, npair], f32)
                nc.vector.memset(zeros32[:], 0.0)
                for k, (didx, par) in enumerate([(0, 0), (1, 0), (0, 1), (1, 1)]):
                    asrc = acc_ps[:].rearrange("p (d s2 two) -> p d s2 two", d=2, two=2)
                    nc.vector.tensor_copy(
                        feats[:, k * npair : (k + 1) * npair],
                        asrc[:, didx, :, par],
                    )
                logit_ps = lpool.tile([1, npair], f32)
                for j in range(8):
                    hps = mpool.tile([P, npair], f32, tag="hps")
                    for k in range(4):
                        nc.tensor.matmul(
                            out=hps[:],
                            lhsT=wmid_sb[:, k * OH + j * P : k * OH + (j + 1) * P],
                            rhs=feats[:, k * npair : (k + 1) * npair],
                            start=(k == 0), stop=(k == 3),
                        )
                    # relu(x + b) on DVE: (hps + bmid_j) max 0
                    hid = apool.tile([P, npair], f32, tag="hid")
                    nc.vector.scalar_tensor_tensor(
                        out=hid[:], in0=hps[:], scalar=bmid_sb[:, j : j + 1],
                        in1=zeros32[:], op0=OP.add, op1=OP.max,
                    )
                    nc.tensor.matmul(
                        out=logit_ps[:],
                        lhsT=wout_sb[:, j : j + 1],
                        rhs=hid[:],
                        start=(j == 0), stop=(j == 7),
                        skip_group_check=True,
                    )
                out_sb = cpool.tile([1, npair], f32)
                nc.scalar.activation(
                    out_sb[:], logit_ps[:], AF.Sigmoid, bias=float(b_out_val)
                )
                nc.sync.dma_start(out=out_d[:], in_=out_sb[:])

    if not nc.is_finalized():
        nc.finalize()
    return nc


def _host_prep(s1, s2, emb_W, W_fwd, b_fwd, W_bwd, b_bwd, W_mid, b_mid, W_out, b_out):
    import ml_dtypes

    bf = ml_dtypes.bfloat16
    s1 = np.asarray(s1); s2 = np.asarray(s2)
    inp = np.concatenate([s1, s2], axis=1).reshape(-1, L).astype(np.int32)  # [512, L]
    lens = (inp != 0).sum(axis=1).astype(np.int32)                          # [512]
    t = np.arange(L)[None, :]
    ridx = np.where(t < lens[:, None], lens[:, None] - 1 - t, t)
    rev = np.take_along_axis(inp, ridx, axis=1)                             # [512, L]

    any_mask = bool((lens < L).any())
    emb = np.ascontiguousarray(np.asarray(emb_W, dtype=np.float32))

    # weights shared by all cores; tanh-form scaling:
    #   sigma slots (i,o,f): wx *1, wh *0.5 (H doubled)
    #   j slot: pre-doubled -> wx *2, wh *1.0
    sx = {0: 1.0, 1: 1.0, 2: 1.0, 3: 2.0}
    sh = {0: 0.5, 1: 0.5, 2: 0.5, 3: 1.0}
    wx = np.empty((P, 2 * 4 * H), dtype=np.float32)
    wh = np.empty((P, 2 * 4 * H), dtype=np.float32)
    for d, Wd in enumerate((W_fwd, W_bwd)):
        Wd = np.asarray(Wd, dtype=np.float32)
        for slot in range(4):
            ref = _SLOT_TO_REF[slot]
            cols = slice(ref * H, (ref + 1) * H)
            wx[:, d * 512 + slot * H : d * 512 + (slot + 1) * H] = Wd[:E, cols] * sx[slot]
            wh[:, d * 512 + slot * H : d * 512 + (slot + 1) * H] = Wd[E:, cols] * sh[slot]
    wx = wx.astype(bf)
    wh = wh.astype(bf)

    # fold mean /L and the H-doubling /2 into W_mid
    Wm = np.asarray(W_mid, dtype=np.float32) / float(2 * L)
    wmid = np.empty((P, 4 * OH), dtype=np.float32)
    for k in range(4):
        wmid[:, k * OH : (k + 1) * OH] = Wm[k * P : (k + 1) * P, :]
    bmid = np.asarray(b_mid, dtype=np.float32).reshape(8, P).T.copy()
    wout = np.asarray(W_out, dtype=np.float32).reshape(8, P).T.copy()

    in_maps = []
    for c in range(NCORES):
        rows = slice(c * G, (c + 1) * G)
        ids = np.empty((P, 2 * NW), dtype=np.int32)
        for d, arr in enumerate((inp[rows], rev[rows])):
            tiles = arr.T.reshape(NW, W * G)  # [tile, 128]
            ids[:, d * NW : (d + 1) * NW] = tiles.T

        lcore = lens[rows]
        om = (np.arange(L)[:, None] >= lcore[None, :]).astype(np.float32)  # [L, G]
        om4 = om.reshape(NW, W * G) * np.float32(-1e9)   # o-slot rows per window
        # full rank-1 row per (dir, window): [i=0 | o=-1e9*mask | f=1 | j=0]
        rowz = np.zeros((NW, 4, W * G), dtype=np.float32)
        rowz[:, 1, :] = om4
        rowz[:, 2, :] = 1.0
        omask = rowz.reshape(1, NW * 4 * W * G).astype(bf)  # shared by both dirs
        in_maps.append({
            "emb": emb, "ids": ids, "wx": wx, "wh": wh, "omask": omask,
            "wmid": wmid, "bmid": bmid, "wout": wout,
        })
    assert not np.any(np.asarray(b_fwd)) and not np.any(np.asarray(b_bwd)), \
        "nonzero LSTM biases not supported by this kernel build"
    return in_maps, any_mask, float(np.asarray(b_out).reshape(-1)[0])


_CACHE = {}


def kernel(**inputs):
    from concourse import bass_utils

    in_maps, any_mask, b_out_val = _host_prep(**inputs)
    key = ("g", any_mask, b_out_val)
    if key not in _CACHE:
        _CACHE[key] = _build_graph(any_mask, b_out_val)
    nc = _CACHE[key]
    res = bass_utils.run_bass_kernel_spmd(
        nc, in_maps, core_ids=list(range(NCORES))
    )
    outs = [np.asarray(res.results[c]["out"]).reshape(-1) for c in range(NCORES)]
    return np.concatenate(outs).astype(np.float32)
